# revision 17
# baseline (speedup 1.0000x reference)
"""Local (sliding-window) MQA attention block on 8 Trainium2 NeuronCores.

Sharding: data-parallel over batch (4) x sequence-parallel over query halves
(2) = 8 cores. Each core computes 1024 query rows of one batch against a
2048-row key halo (window=1024), all 16 query heads, with the single shared
KV head replicated. Outputs are disjoint row-slices of the final projection,
so no cross-core collectives are needed.

All matmul operands are bf16 (fp32 PSUM accumulation). Weights and x are
pre-packed host-side into the exact SBUF layouts so every DMA is
partition-contiguous. x^T is SBUF-resident (local half) so the q-projection
never waits on DMA. Attention runs in transposed layout (S^T = k^T.T @ q^T)
with per-slot column trimming from the causal/window structure: only two
static 128x128 triangle masks are ever applied (on the PE, fused into the
S accumulation); halo padding is handled by a per-slot exp bias.
"""
import sys

for _p in ("/opt/trn_rl_repo",):
    if _p not in sys.path:
        sys.path.insert(0, _p)

import ml_dtypes
import numpy as np

import concourse.bass as bass
import concourse.bacc as bacc
import concourse.tile as tile
import concourse.mybir as mybir
from concourse.bass_utils import run_bass_kernel_spmd

F32 = mybir.dt.float32
F32R = mybir.dt.float32r
BF16 = mybir.dt.bfloat16
EXP = mybir.ActivationFunctionType.Exp
NPBF = ml_dtypes.bfloat16

B, T, W = 4, 2048, 2048
NH, HD = 16, 128
WIN = 1024
QL = 1024          # query rows per core
KB = 2048          # key-halo rows per core
QBS = 512          # query block (moving free dim)
NQB = QL // QBS    # 2 query blocks per core
SLOTS = (WIN + QBS) // 128  # 12 key slots of 128 per query block
NEG = -1.0e9
SCALE = HD ** -0.5
MAX_WAVELENGTH = 10000.0
NW = W // 128      # 16 width chunks

# Per-slot trimmed column ranges [c0, c1) within the 512-query block, the
# per-slot triangle mask (None / 'up' / 'lo'), and the emission order (the
# first emitted slot must span the full [0, 512) so PSUM accumulation of
# the denominator / PV starts on the whole range).
#   slots 0-3  (window left edge): cols [0, 128*(k+1)), upper-NEG triangle
#                                  at the last 128 cols
#   slots 4-7  (interior):         full, no mask
#   slots 8-11 (causal edge):      cols [128*(k-8), 512), lower-NEG triangle
#                                  at the first 128 cols
SLOT_TRIM = {}
for _k in range(SLOTS):
    if _k < 4:
        SLOT_TRIM[_k] = (0, 128 * (_k + 1), "up")
    elif _k < 8:
        SLOT_TRIM[_k] = (0, 512, None)
    else:
        SLOT_TRIM[_k] = (128 * (_k - 8), 512, "lo")
SLOT_ORDER = [8, 3, 0, 1, 2, 4, 5, 6, 7, 9, 10, 11]

_COMPILED = None


def _rope_tables(pos):
    """pos: [n] int -> (cmul, smul) [128, n] such that
    rope(x)[d] = x[d]*cmul[d] + x[shuf(d)]*smul[d], shuf(d)=d^32 for d<64."""
    half = 32
    inv_freq = MAX_WAVELENGTH ** (-(2.0 * np.arange(half, dtype=np.float64)) / 64.0)
    ang = pos.astype(np.float64)[None, :] * inv_freq[:, None]   # [32, n]
    sin, cos = np.sin(ang), np.cos(ang)
    n = pos.shape[0]
    cmul = np.ones((HD, n), dtype=np.float64)
    smul = np.zeros((HD, n), dtype=np.float64)
    cmul[0:32] = cos
    cmul[32:64] = cos
    smul[0:32] = -sin
    smul[32:64] = sin
    return cmul.astype(NPBF), smul.astype(NPBF)


def _emit_rope(nc, pool, dst, src_ps, cmul, smul, n):
    """dst[0:64] = src[0:64]*c[0:64] + shuf(src)[0:64]*s[0:64]; dst[64:128]=src.
    dst: SBUF bf16 AP [128, n]; src_ps: PSUM f32 AP [128, n]; cmul/smul bf16.
    Partition-shuffle + pass-through copies run on ScalarE to offload DVE."""
    t1 = pool.tile([64, n], BF16, tag="rope_t1", bufs=2)
    t2 = pool.tile([64, n], BF16, tag="rope_t2", bufs=2)
    stage = pool.tile([64, n], BF16, tag="rope_stage", bufs=2)
    nc.vector.tensor_mul(t1[:, :], src_ps[0:64, :], cmul[0:64, :])
    nc.scalar.copy(out=stage[0:32, :], in_=src_ps[32:64, :])
    nc.scalar.copy(out=stage[32:64, :], in_=src_ps[0:32, :])
    nc.vector.tensor_mul(t2[:, :], stage[:, :], smul[0:64, :])
    nc.vector.tensor_add(dst[0:64, :], t1[:, :], t2[:, :])
    nc.scalar.copy(out=dst[64:128, :], in_=src_ps[64:128, :])


def _build_program():
    nc = bacc.Bacc("TRN2", target_bir_lowering=False, debug=False)

    xtl = nc.dram_tensor("xtl", [NW, 128, QL], BF16, kind="ExternalInput")
    xth = nc.dram_tensor("xth", [NW, 128, QL], BF16, kind="ExternalInput")
    wqp = nc.dram_tensor("wqp", [4, 128, NW, 512], BF16, kind="ExternalInput")
    wkp = nc.dram_tensor("wkp", [128, NW, HD], BF16, kind="ExternalInput")
    wvp = nc.dram_tensor("wvp", [128, NW, HD], BF16, kind="ExternalInput")
    wop = nc.dram_tensor("wop", [4, 128, NH, 512], BF16, kind="ExternalInput")
    bias = nc.dram_tensor("bias", [W], F32, kind="ExternalInput")
    cq_d = nc.dram_tensor("cq", [HD, QL], BF16, kind="ExternalInput")
    sq_d = nc.dram_tensor("sq", [HD, QL], BF16, kind="ExternalInput")
    ck_d = nc.dram_tensor("ck", [HD, KB], BF16, kind="ExternalInput")
    sk_d = nc.dram_tensor("sk", [HD, KB], BF16, kind="ExternalInput")
    tri_d = nc.dram_tensor("tri", [2, 128, 128], BF16, kind="ExternalInput")
    ident_d = nc.dram_tensor("ident", [128, 128], BF16, kind="ExternalInput")
    ones_d = nc.dram_tensor("ones", [128, 1], F32R, kind="ExternalInput")
    padb_d = nc.dram_tensor("padb", [128, KB // 128], F32, kind="ExternalInput")
    out = nc.dram_tensor("out", [QL, W], F32, kind="ExternalOutput")

    with tile.TileContext(nc) as tc:
        with tc.tile_pool(name="persist", bufs=1) as pp:
            ones_sb = pp.tile([128, 1], F32R, tag="ones")
            ident = pp.tile([128, 128], BF16, tag="ident")
            tri = pp.tile([128, 2, 128], BF16, tag="tri")
            bias_bc = pp.tile([128, W], F32, tag="biasbc")
            padb = pp.tile([128, KB // 128], F32, tag="padb")

            nc.gpsimd.dma_start(out=ones_sb[:, :], in_=ones_d[:, :])
            nc.gpsimd.dma_start(out=ident[:, :], in_=ident_d[:, :])
            for u in range(2):
                nc.gpsimd.dma_start(out=tri[:, u, :], in_=tri_d[u, :, :])
            nc.gpsimd.dma_start(out=padb[:, :], in_=padb_d[:, :])
            b_ap = bias.ap()
            nc.gpsimd.dma_start(out=bias_bc[:, :], in_=bass.AP(
                tensor=b_ap.tensor, offset=b_ap.offset,
                ap=[[0, 128]] + list(b_ap.ap)))

            # PE p-state warm-up: harmless matmuls as soon as ident lands so
            # the array is at full clock when the first real matmul issues
            with tc.tile_pool(name="warm", bufs=1, space="PSUM") as pwm:
                wps = pwm.tile([128, 128], F32, tag="warm")
                for _ in range(16):
                    nc.tensor.matmul(out=wps[:, :], lhsT=ident[:, :],
                                     rhs=ident[:, :], start=True, stop=True)

            with tc.tile_pool(name="p1", bufs=1) as p1, \
                 tc.tile_pool(name="pr", bufs=3) as pr:
                kT_sb = p1.tile([HD, KB], BF16, tag="kT")     # rope'd k^T
                v_sb = p1.tile([128, KB], F32R, tag="v")      # natural v
                qT_sb = p1.tile([HD, NH, QL], BF16, tag="qT")  # rope'd q^T
                cq = p1.tile([HD, QL], BF16, tag="cq")
                sq = p1.tile([HD, QL], BF16, tag="sq")
                ck = p1.tile([HD, KB], BF16, tag="ck")
                sk = p1.tile([HD, KB], BF16, tag="sk")
                nc.gpsimd.dma_start(out=ck[:, :], in_=ck_d[:, :])
                nc.gpsimd.dma_start(out=sk[:, :], in_=sk_d[:, :])
                nc.gpsimd.dma_start(out=cq[:, :], in_=cq_d[:, :])
                nc.gpsimd.dma_start(out=sq[:, :], in_=sq_d[:, :])

                # ============ Phases A+B: projections ============
                with tc.tile_pool(name="px", bufs=1) as px:
                    xt_sb = px.tile([128, NW, QL], BF16, tag="xt")
                    wk_sb = px.tile([128, NW, HD], BF16, tag="wk")
                    wv_sb = px.tile([128, NW, HD], BF16, tag="wv")
                    nc.scalar.dma_start(out=wk_sb[:, :, :], in_=wkp.ap())
                    nc.gpsimd.dma_start(out=wv_sb[:, :, :], in_=wvp.ap())

                    # ---- Phase A: k/v projections over the halo ----
                    # (wave-0 wq prefetched here so phase B starts instantly;
                    #  resident-x chunk DMAs are interleaved with the halo
                    #  stream so the first halo chunks aren't queued behind
                    #  the 8 MiB bulk load)
                    pbw_cm = tc.tile_pool(name="pbw", bufs=2)
                    pbw = pbw_cm.__enter__()
                    wq_w0 = pbw.tile([128, NW, 512], BF16, tag="wqw")
                    nc.scalar.dma_start(out=wq_w0[:, :, :],
                                        in_=wqp[0, :, :, :])
                    with tc.tile_pool(name="pah", bufs=3) as pah, \
                         tc.tile_pool(name="pa_ps", bufs=2,
                                      space="PSUM") as paps:
                        for sq2 in range(2):
                            kt_ps = [paps.tile([HD, 512], F32, tag="kt_ps",
                                               name="kt_ps") for _ in range(2)]
                            vt_ps = [paps.tile([HD, 512], F32, tag="vt_ps",
                                               name="vt_ps") for _ in range(2)]
                            for wc in range(NW):
                                if sq2 == 0:
                                    xsrc = pah.tile([128, QL], BF16, tag="xh")
                                    nc.sync.dma_start(out=xsrc[:, :],
                                                      in_=xth[wc, :, :])
                                    nc.gpsimd.dma_start(
                                        out=xt_sb[:, wc, :],
                                        in_=xtl[wc, :, :])
                                else:
                                    xsrc = xt_sb[:, wc, :]
                                for hf in range(2):
                                    nc.tensor.matmul(
                                        out=kt_ps[hf][:, :],
                                        lhsT=wk_sb[:, wc, :],
                                        rhs=xsrc[:, QBS * hf:QBS * (hf + 1)],
                                        start=(wc == 0), stop=(wc == NW - 1))
                                    nc.tensor.matmul(
                                        out=vt_ps[hf][:, :],
                                        lhsT=wv_sb[:, wc, :],
                                        rhs=xsrc[:, QBS * hf:QBS * (hf + 1)],
                                        start=(wc == 0), stop=(wc == NW - 1))
                            for hf in range(2):
                                sq4 = 2 * sq2 + hf
                                cols = slice(512 * sq4, 512 * (sq4 + 1))
                                _emit_rope(nc, pr, kT_sb[:, cols],
                                           kt_ps[hf][:, :],
                                           ck[:, cols], sk[:, cols], 512)
                                # v: PSUM->SBUF copy + PE-transpose 128-blocks
                                vt_sb = pr.tile([HD, 512], BF16, tag="vt_sb")
                                nc.vector.tensor_copy(out=vt_sb[:, :],
                                                      in_=vt_ps[hf][:, :])
                                for j in range(4):
                                    vps2 = paps.tile([128, 128], BF16,
                                                     tag="vT2")
                                    nc.tensor.transpose(
                                        vps2[:, :],
                                        vt_sb[:, 128 * j:128 * (j + 1)],
                                        ident[:, :])
                                    blk = 4 * sq4 + j
                                    nc.vector.tensor_copy(
                                        out=v_sb[:, 128 * blk:128 * (blk + 1)],
                                        in_=vps2[:, :])

                    # ---- Phase B: q projections, 4 waves of 4 heads ----
                    with tc.tile_pool(name="pb_ps", bufs=8,
                                      space="PSUM") as pbps:
                        wq_next = wq_w0
                        for wave in range(4):
                            wq_w = wq_next
                            if wave < 3:
                                wq_next = pbw.tile([128, NW, 512], BF16,
                                                   tag="wqw")
                                nc.scalar.dma_start(out=wq_next[:, :, :],
                                                    in_=wqp[wave + 1, :, :, :])
                            q_ps = [[pbps.tile([HD, QBS], F32, tag="q_ps",
                                               name="q_ps")
                                     for _ in range(2)] for _ in range(4)]
                            for wc in range(NW):
                                for j4 in range(4):
                                    for qh in range(2):
                                        nc.tensor.matmul(
                                            out=q_ps[j4][qh][:, :],
                                            lhsT=wq_w[:, wc,
                                                      128 * j4:128 * (j4 + 1)],
                                            rhs=xt_sb[:, wc,
                                                      QBS * qh:QBS * (qh + 1)],
                                            start=(wc == 0),
                                            stop=(wc == NW - 1))
                            for j4 in range(4):
                                head = wave * 4 + j4
                                for qh in range(2):
                                    _emit_rope(
                                        nc, pr,
                                        qT_sb[:, head, QBS * qh:QBS * (qh + 1)],
                                        q_ps[j4][qh][:, :],
                                        cq[:, QBS * qh:QBS * (qh + 1)],
                                        sq[:, QBS * qh:QBS * (qh + 1)], QBS)
                    pbw_cm.__exit__(None, None, None)

                # ============ Phases C+D ============
                with tc.tile_pool(name="pcd", bufs=1) as pcd, \
                     tc.tile_pool(name="pet", bufs=8) as pet, \
                     tc.tile_pool(name="pcn", bufs=3) as pcn:
                    encT = pcd.tile([HD, NH, QL], BF16, tag="encT")
                    wot_sb = pcd.tile([128, 4, NH, 512], BF16, tag="wot")
                    for oc in range(4):
                        nc.scalar.dma_start(out=wot_sb[:, oc, :, :],
                                            in_=wop[oc, :, :, :])

                    # ---- Phase C: attention ----
                    with tc.tile_pool(name="pc_s", bufs=4,
                                      space="PSUM") as pcs, \
                         tc.tile_pool(name="pc_a", bufs=2,
                                      space="PSUM") as pca:
                        for i in range(NQB):
                            for head in range(NH):
                                qs = qT_sb[:, head, QBS * i:QBS * (i + 1)]
                                enc_ps = pca.tile([HD, QBS], F32, tag="enc_ps")
                                den_ps = pca.tile([1, QBS], F32, tag="den_ps")
                                ets = [None] * SLOTS

                                def emit_s(k):
                                    c0, c1, msk = SLOT_TRIM[k]
                                    w = c1 - c0
                                    s_ps = pcs.tile([128, QBS], F32,
                                                    tag="s_ps")
                                    kcol = 512 * i + 128 * k
                                    nc.tensor.matmul(
                                        out=s_ps[:, 0:w],
                                        lhsT=kT_sb[:, kcol:kcol + 128],
                                        rhs=qs[:, c0:c1],
                                        start=True, stop=msk is None)
                                    if msk is not None:
                                        toff = w - 128 if msk == "up" else 0
                                        nc.tensor.matmul(
                                            out=s_ps[:, toff:toff + 128],
                                            lhsT=ident[:, :],
                                            rhs=tri[:, 0 if msk == "up" else 1, :],
                                            start=False, stop=True,
                                            skip_group_check=True)
                                    blk = 4 * i + k
                                    # f32r et: ACT writes 16-bit outputs at
                                    # ~2/3 throughput, and f32r moving
                                    # operands >=256 cols run at full PE
                                    # speed anyway. Narrow (128-col) slots
                                    # stay bf16 to dodge the f32r 4x penalty.
                                    if w >= 256:
                                        et = pet.tile([128, QBS], F32R,
                                                      tag="et_w")
                                    else:
                                        et = pet.tile([128, 128], F32R,
                                                      tag="et_n", bufs=4)
                                    nc.scalar.activation(
                                        out=et[:, 0:w], in_=s_ps[:, 0:w],
                                        func=EXP,
                                        bias=padb[:, blk:blk + 1])
                                    ets[k] = et

                                def emit_acc(k, first, last):
                                    c0, c1, _ = SLOT_TRIM[k]
                                    w = c1 - c0
                                    blk = 4 * i + k
                                    nc.tensor.matmul(
                                        out=den_ps[:, c0:c1],
                                        lhsT=ones_sb[:, :],
                                        rhs=ets[k][:, 0:w],
                                        start=first, stop=last,
                                        skip_group_check=True)
                                    nc.tensor.matmul(
                                        out=enc_ps[:, c0:c1],
                                        lhsT=v_sb[:, 128 * blk:128 * (blk + 1)],
                                        rhs=ets[k][:, 0:w],
                                        start=first, stop=last,
                                        skip_group_check=True)

                                # software pipeline: S emitted LA slots ahead
                                LA = 4
                                for j in range(LA):
                                    emit_s(SLOT_ORDER[j])
                                for j in range(LA, SLOTS):
                                    emit_s(SLOT_ORDER[j])
                                    emit_acc(SLOT_ORDER[j - LA],
                                             j - LA == 0, False)
                                for j in range(SLOTS - LA, SLOTS):
                                    emit_acc(SLOT_ORDER[j], False,
                                             j == SLOTS - 1)

                                den_sb = pcn.tile([1, QBS], F32, tag="den_sb")
                                nc.vector.reciprocal_approx_fast(
                                    out=den_sb[:, :], in_=den_ps[:, :])
                                den_bc = pcn.tile([128, QBS], F32, tag="den_bc")
                                nc.gpsimd.partition_broadcast(
                                    den_bc[:, :], den_sb[:, :])
                                nc.vector.tensor_mul(
                                    encT[:, head, QBS * i:QBS * (i + 1)],
                                    enc_ps[:, :], den_bc[:, :])

                    # ---- Phase D: output projection ----
                    with tc.tile_pool(name="pdo", bufs=3) as pdo, \
                         tc.tile_pool(name="pd_ps", bufs=4,
                                      space="PSUM") as pdps:
                        for oc in range(4):
                            for tsub in range(QL // 128):
                                o_ps = pdps.tile([128, 512], F32, tag="o_ps")
                                for n in range(NH):
                                    nc.tensor.matmul(
                                        out=o_ps[:, :],
                                        lhsT=encT[:, n,
                                                  128 * tsub:128 * (tsub + 1)],
                                        rhs=wot_sb[:, oc, n, :],
                                        start=(n == 0), stop=(n == NH - 1))
                                o_sb = pdo.tile([128, 512], F32, tag="o_sb")
                                nc.vector.tensor_add(
                                    o_sb[:, :], o_ps[:, :],
                                    bias_bc[:, 512 * oc:512 * (oc + 1)])
                                nc.sync.dma_start(
                                    out=out[128 * tsub:128 * (tsub + 1),
                                            512 * oc:512 * (oc + 1)],
                                    in_=o_sb[:, :])

    nc.compile()
    return nc


def _get_program():
    global _COMPILED
    if _COMPILED is None:
        _COMPILED = _build_program()
    return _COMPILED


def _check_mask(attention_mask):
    """This kernel is specialized to the causal + sliding-window mask."""
    am = np.asarray(attention_mask)
    t = np.arange(T)[:, None]
    s = np.arange(T)[None, :]
    expect = (t >= s) & (t <= s + WIN)
    if am.shape != (T, T) or not np.array_equal(am, expect):
        raise ValueError(
            "attention_mask does not match the causal+window(1024) structure "
            "this kernel is specialized for")


def _prep_core_inputs(x, segment_pos, shared):
    """Per-core input dicts. Core c: batch c//2, query half c%2."""
    segment_pos = np.asarray(segment_pos)
    in_maps = []
    for c in range(8):
        b, h = c // 2, c % 2
        key_start = QL * h - WIN
        xb = x[b].T.astype(NPBF)                 # [W, T]
        # local half: query rows owned by this core, chunked by width
        xtl = np.ascontiguousarray(
            xb[:, QL * h:QL * (h + 1)].reshape(NW, 128, QL))
        # halo half: key rows [key_start, key_start + QL), zero-padded
        xth = np.zeros((NW, 128, QL), dtype=NPBF)
        lo = max(0, -key_start)
        if lo < QL:
            xth[:, :, lo:] = xb[:, key_start + lo:key_start + QL] \
                .reshape(NW, 128, QL - lo)

        g_q = QL * h + np.arange(QL)                      # global query rows
        g_k = key_start + np.arange(KB)                   # global key rows
        pos_q = segment_pos[g_q]
        pos_k = np.where((g_k >= 0) & (g_k < T),
                         segment_pos[np.clip(g_k, 0, T - 1)], 0)
        cq, sq = _rope_tables(pos_q)
        ck, sk = _rope_tables(pos_k)

        ok_k = (g_k >= 0) & (g_k < T)
        padb = np.ascontiguousarray(np.where(
            ok_k, 0.0, NEG).astype(np.float32).reshape(KB // 128, 128).T)
        in_maps.append(dict(shared, xtl=xtl, xth=xth, cq=cq, sq=sq,
                            ck=ck, sk=sk, padb=padb))
    return in_maps


def _pack_w(wt):
    """[W, 2048] (transposed weight, contraction-major) ->
    [4, 128, 16, 512] stationary-chunk layout, bf16."""
    return np.ascontiguousarray(
        wt.reshape(NW, 128, 4, 512).transpose(2, 1, 0, 3)).astype(NPBF)


def kernel(x, segment_pos, attention_mask, wq, wk, wv, w_out, b_out):
    x = np.asarray(x, dtype=np.float32)
    wq = np.asarray(wq, dtype=np.float32)
    wk = np.asarray(wk, dtype=np.float32)
    wv = np.asarray(wv, dtype=np.float32)
    w_out = np.asarray(w_out, dtype=np.float32)
    b_out = np.asarray(b_out, dtype=np.float32)

    _check_mask(attention_mask)

    nc = _get_program()
    dt = np.arange(128)
    tri_up = np.where(dt[None, :] > dt[:, None], NEG, 0.0).astype(NPBF)
    tri_lo = np.where(dt[:, None] > dt[None, :], NEG, 0.0).astype(NPBF)
    shared = {
        "wqp": _pack_w(wq.T * np.float32(SCALE)),
        "wkp": np.ascontiguousarray(
            wk.T.reshape(NW, 128, HD).transpose(1, 0, 2)).astype(NPBF),
        "wvp": np.ascontiguousarray(
            wv.T.reshape(NW, 128, HD).transpose(1, 0, 2)).astype(NPBF),
        "wop": _pack_w(w_out.T),
        "bias": b_out,
        "tri": np.stack([tri_up, tri_lo]),
        "ident": np.eye(128, dtype=np.float32).astype(NPBF),
        "ones": np.ones((128, 1), dtype=np.float32),
    }
    in_maps = _prep_core_inputs(x, segment_pos, shared)
    res = run_bass_kernel_spmd(nc, in_maps, list(range(8)))
    global _LAST_RESULT
    _LAST_RESULT = res

    out = np.empty((B, T, W), dtype=np.float32)
    for c in range(8):
        b, h = c // 2, c % 2
        out[b, QL * h:QL * (h + 1), :] = res.results[c]["out"]
    return out


# revision 22
# speedup vs baseline: 1.0673x; 1.0673x over previous
"""Local (sliding-window) MQA attention block on 8 Trainium2 NeuronCores.

Sharding: data-parallel over batch (4) x sequence-parallel over query halves
(2) = 8 cores. Each core computes 1024 query rows of one batch against a
2048-row key halo (window=1024), all 16 query heads, with the single shared
KV head replicated. Outputs are disjoint row-slices of the final projection,
so no cross-core collectives are needed.

All matmul operands are bf16 (fp32 PSUM accumulation). Weights and x are
pre-packed host-side into the exact SBUF layouts so every DMA is
partition-contiguous. x^T is SBUF-resident (local half) so the q-projection
never waits on DMA. Attention runs in transposed layout (S^T = k^T.T @ q^T)
with per-slot column trimming from the causal/window structure: only two
static 128x128 triangle masks are ever applied (on the PE, fused into the
S accumulation); halo padding is handled by a per-slot exp bias.
"""
import sys

for _p in ("/opt/trn_rl_repo",):
    if _p not in sys.path:
        sys.path.insert(0, _p)

import ml_dtypes
import numpy as np

import concourse.bass as bass
import concourse.bacc as bacc
import concourse.tile as tile
import concourse.mybir as mybir
from concourse.bass_utils import run_bass_kernel_spmd

F32 = mybir.dt.float32
F32R = mybir.dt.float32r
BF16 = mybir.dt.bfloat16
EXP = mybir.ActivationFunctionType.Exp
NPBF = ml_dtypes.bfloat16

B, T, W = 4, 2048, 2048
NH, HD = 16, 128
WIN = 1024
QL = 1024          # query rows per core
KB = 2048          # key-halo rows per core
QBS = 512          # query block (moving free dim)
NQB = QL // QBS    # 2 query blocks per core
SLOTS = (WIN + QBS) // 128  # 12 key slots of 128 per query block
NEG = -1.0e9
SCALE = HD ** -0.5
MAX_WAVELENGTH = 10000.0
NW = W // 128      # 16 width chunks

# Per-slot trimmed column ranges [c0, c1) within the 512-query block, the
# per-slot triangle mask (None / 'up' / 'lo'), and the emission order (the
# first emitted slot must span the full [0, 512) so PSUM accumulation of
# the denominator / PV starts on the whole range).
#   slots 0-3  (window left edge): cols [0, 128*(k+1)), upper-NEG triangle
#                                  at the last 128 cols
#   slots 4-7  (interior):         full, no mask
#   slots 8-11 (causal edge):      cols [128*(k-8), 512), lower-NEG triangle
#                                  at the first 128 cols
SLOT_TRIM = {}
for _k in range(SLOTS):
    if _k < 4:
        SLOT_TRIM[_k] = (0, 128 * (_k + 1), "up")
    elif _k < 8:
        SLOT_TRIM[_k] = (0, 512, None)
    else:
        SLOT_TRIM[_k] = (128 * (_k - 8), 512, "lo")
SLOT_ORDER = [8, 3, 0, 1, 2, 4, 5, 6, 7, 9, 10, 11]

_COMPILED = None


def _rope_tables(pos):
    """pos: [n] int -> (cmul, smul) [128, n] such that
    rope(x)[d] = x[d]*cmul[d] + x[shuf(d)]*smul[d], shuf(d)=d^32 for d<64."""
    half = 32
    inv_freq = MAX_WAVELENGTH ** (-(2.0 * np.arange(half, dtype=np.float64)) / 64.0)
    ang = pos.astype(np.float64)[None, :] * inv_freq[:, None]   # [32, n]
    sin, cos = np.sin(ang), np.cos(ang)
    n = pos.shape[0]
    cmul = np.ones((HD, n), dtype=np.float64)
    smul = np.zeros((HD, n), dtype=np.float64)
    cmul[0:32] = cos
    cmul[32:64] = cos
    smul[0:32] = -sin
    smul[32:64] = sin
    return cmul.astype(NPBF), smul.astype(NPBF)


def _emit_rope(nc, pool, dst, src_ps, cmul, smul, n):
    """dst[0:64] = src[0:64]*c[0:64] + shuf(src)[0:64]*s[0:64]; dst[64:128]=src.
    dst: SBUF bf16 AP [128, n]; src_ps: PSUM f32 AP [128, n]; cmul/smul bf16.
    Partition-shuffle + pass-through copies run on ScalarE to offload DVE."""
    t1 = pool.tile([64, n], BF16, tag="rope_t1", bufs=2)
    t2 = pool.tile([64, n], BF16, tag="rope_t2", bufs=2)
    stage = pool.tile([64, n], BF16, tag="rope_stage", bufs=2)
    nc.vector.tensor_mul(t1[:, :], src_ps[0:64, :], cmul[0:64, :])
    nc.scalar.copy(out=stage[0:32, :], in_=src_ps[32:64, :])
    nc.scalar.copy(out=stage[32:64, :], in_=src_ps[0:32, :])
    nc.vector.tensor_mul(t2[:, :], stage[:, :], smul[0:64, :])
    nc.vector.tensor_add(dst[0:64, :], t1[:, :], t2[:, :])
    nc.scalar.copy(out=dst[64:128, :], in_=src_ps[64:128, :])


def _build_program():
    nc = bacc.Bacc("TRN2", target_bir_lowering=False, debug=False)

    xtc = nc.dram_tensor("xtc", [NW, 128, KB], BF16, kind="ExternalInput")
    wqp = nc.dram_tensor("wqp", [4, 128, NW, 512], BF16, kind="ExternalInput")
    wkp = nc.dram_tensor("wkp", [128, NW, HD], BF16, kind="ExternalInput")
    wvp = nc.dram_tensor("wvp", [128, NW, HD], BF16, kind="ExternalInput")
    wop = nc.dram_tensor("wop", [4, 128, NH, 512], BF16, kind="ExternalInput")
    bias = nc.dram_tensor("bias", [W], F32, kind="ExternalInput")
    cq_d = nc.dram_tensor("cq", [HD, QL], BF16, kind="ExternalInput")
    sq_d = nc.dram_tensor("sq", [HD, QL], BF16, kind="ExternalInput")
    ck_d = nc.dram_tensor("ck", [HD, KB], BF16, kind="ExternalInput")
    sk_d = nc.dram_tensor("sk", [HD, KB], BF16, kind="ExternalInput")
    tri_d = nc.dram_tensor("tri", [2, 128, 128], BF16, kind="ExternalInput")
    ident_d = nc.dram_tensor("ident", [128, 128], BF16, kind="ExternalInput")
    ones_d = nc.dram_tensor("ones", [128, 1], F32R, kind="ExternalInput")
    padb_d = nc.dram_tensor("padb", [128, KB // 128], F32, kind="ExternalInput")
    out = nc.dram_tensor("out", [QL, W], F32, kind="ExternalOutput")

    with tile.TileContext(nc) as tc:
        with tc.tile_pool(name="persist", bufs=1) as pp:
            ones_sb = pp.tile([128, 1], F32R, tag="ones")
            ident = pp.tile([128, 128], BF16, tag="ident")
            tri = pp.tile([128, 2, 128], BF16, tag="tri")
            bias_bc = pp.tile([128, W], F32, tag="biasbc")
            padb = pp.tile([128, KB // 128], F32, tag="padb")

            nc.sync.dma_start(out=ones_sb[:, :], in_=ones_d[:, :])
            nc.sync.dma_start(out=ident[:, :], in_=ident_d[:, :])
            for u in range(2):
                nc.sync.dma_start(out=tri[:, u, :], in_=tri_d[u, :, :])
            nc.gpsimd.dma_start(out=padb[:, :], in_=padb_d[:, :])

            # PE p-state warm-up: harmless matmuls as soon as ident lands so
            # the array is at full clock when the first real matmul issues
            with tc.tile_pool(name="warm", bufs=1, space="PSUM") as pwm:
                wps = pwm.tile([128, 128], F32, tag="warm")
                for _ in range(16):
                    nc.tensor.matmul(out=wps[:, :], lhsT=ident[:, :],
                                     rhs=ident[:, :], start=True, stop=True)

            with tc.tile_pool(name="p1", bufs=1) as p1, \
                 tc.tile_pool(name="pr", bufs=3) as pr:
                kT_sb = p1.tile([HD, KB], BF16, tag="kT")     # rope'd k^T
                v_sb = p1.tile([128, KB], F32R, tag="v")      # natural v
                qT_sb = p1.tile([HD, NH, QL], BF16, tag="qT")  # rope'd q^T
                cq = p1.tile([HD, QL], BF16, tag="cq")
                sq = p1.tile([HD, QL], BF16, tag="sq")
                ck = p1.tile([HD, KB], BF16, tag="ck")
                sk = p1.tile([HD, KB], BF16, tag="sk")
                nc.scalar.dma_start(out=ck[:, :], in_=ck_d[:, :])
                nc.scalar.dma_start(out=sk[:, :], in_=sk_d[:, :])
                nc.scalar.dma_start(out=cq[:, :], in_=cq_d[:, :])
                nc.scalar.dma_start(out=sq[:, :], in_=sq_d[:, :])

                # ============ Phases A+B: projections ============
                # x^T is fully SBUF-resident ([halo | local] columns).
                # All contraction (wc) loops are INNER: one PSUM accumulator
                # chain at a time, so only ~3 PSUM banks are live and each
                # rope starts right after its chain - the rope pipeline
                # drains with the matmuls instead of trailing the phase.
                with tc.tile_pool(name="px", bufs=1) as px:
                    xt_sb = px.tile([128, NW, KB], BF16, tag="xt")
                    wk_sb = px.tile([128, NW, HD], BF16, tag="wk")
                    wv_sb = px.tile([128, NW, HD], BF16, tag="wv")
                    nc.scalar.dma_start(out=wk_sb[:, :, :], in_=wkp.ap())
                    nc.gpsimd.dma_start(out=wv_sb[:, :, :], in_=wvp.ap())
                    # halo halves first (phase A consumes them first)
                    for half in range(2):
                        for wc in range(NW):
                            eng = nc.sync if wc % 2 == 0 else nc.gpsimd
                            eng.dma_start(
                                out=xt_sb[:, wc, QL * half:QL * (half + 1)],
                                in_=xtc[wc, :, QL * half:QL * (half + 1)])

                    pbw_cm = tc.tile_pool(name="pbw", bufs=2)
                    pbw = pbw_cm.__enter__()
                    wq_w0 = pbw.tile([128, NW, 512], BF16, tag="wqw")
                    nc.scalar.dma_start(out=wq_w0[:, :, :],
                                        in_=wqp[0, :, :, :])

                    # ---- Phase A: k/v projections over the halo ----
                    with tc.tile_pool(name="pa_ps", bufs=3,
                                      space="PSUM") as paps:
                        for sq4 in range(4):
                            cols = slice(512 * sq4, 512 * (sq4 + 1))
                            kt_ps = paps.tile([HD, 512], F32, tag="a_ps")
                            for wc in range(NW):
                                nc.tensor.matmul(
                                    out=kt_ps[:, :], lhsT=wk_sb[:, wc, :],
                                    rhs=xt_sb[:, wc, cols],
                                    start=(wc == 0), stop=(wc == NW - 1))
                            _emit_rope(nc, pr, kT_sb[:, cols], kt_ps[:, :],
                                       ck[:, cols], sk[:, cols], 512)
                            vt_ps = paps.tile([HD, 512], F32, tag="a_ps")
                            for wc in range(NW):
                                nc.tensor.matmul(
                                    out=vt_ps[:, :], lhsT=wv_sb[:, wc, :],
                                    rhs=xt_sb[:, wc, cols],
                                    start=(wc == 0), stop=(wc == NW - 1))
                            # v: PSUM->SBUF copy + PE-transpose 128-blocks
                            vt_sb = pr.tile([HD, 512], BF16, tag="vt_sb")
                            nc.vector.tensor_copy(out=vt_sb[:, :],
                                                  in_=vt_ps[:, :])
                            for j in range(4):
                                vps2 = paps.tile([128, 128], BF16,
                                                 tag="vT2")
                                nc.tensor.transpose(
                                    vps2[:, :],
                                    vt_sb[:, 128 * j:128 * (j + 1)],
                                    ident[:, :])
                                blk = 4 * sq4 + j
                                nc.vector.tensor_copy(
                                    out=v_sb[:, 128 * blk:128 * (blk + 1)],
                                    in_=vps2[:, :])

                    # ---- Phase B: q projections, 4 waves of 4 heads ----
                    with tc.tile_pool(name="pb_ps", bufs=3,
                                      space="PSUM") as pbps:
                        wq_next = wq_w0
                        for wave in range(4):
                            wq_w = wq_next
                            if wave < 3:
                                wq_next = pbw.tile([128, NW, 512], BF16,
                                                   tag="wqw")
                                nc.scalar.dma_start(out=wq_next[:, :, :],
                                                    in_=wqp[wave + 1, :, :, :])
                            for j4 in range(4):
                                head = wave * 4 + j4
                                for qh in range(2):
                                    q_ps = pbps.tile([HD, QBS], F32,
                                                     tag="q_ps")
                                    for wc in range(NW):
                                        nc.tensor.matmul(
                                            out=q_ps[:, :],
                                            lhsT=wq_w[:, wc,
                                                      128 * j4:128 * (j4 + 1)],
                                            rhs=xt_sb[:, wc,
                                                      QL + QBS * qh:
                                                      QL + QBS * (qh + 1)],
                                            start=(wc == 0),
                                            stop=(wc == NW - 1))
                                    _emit_rope(
                                        nc, pr,
                                        qT_sb[:, head, QBS * qh:QBS * (qh + 1)],
                                        q_ps[:, :],
                                        cq[:, QBS * qh:QBS * (qh + 1)],
                                        sq[:, QBS * qh:QBS * (qh + 1)], QBS)
                    pbw_cm.__exit__(None, None, None)

                # ============ Phases C+D ============
                with tc.tile_pool(name="pcd", bufs=1) as pcd, \
                     tc.tile_pool(name="pet", bufs=8) as pet, \
                     tc.tile_pool(name="pcn", bufs=3) as pcn:
                    encT = pcd.tile([HD, NH, QL], BF16, tag="encT")
                    wot_sb = pcd.tile([128, 4, NH, 512], BF16, tag="wot")
                    b_ap = bias.ap()
                    nc.gpsimd.dma_start(out=bias_bc[:, :], in_=bass.AP(
                        tensor=b_ap.tensor, offset=b_ap.offset,
                        ap=[[0, 128]] + list(b_ap.ap)))
                    for oc in range(4):
                        nc.scalar.dma_start(out=wot_sb[:, oc, :, :],
                                            in_=wop[oc, :, :, :])

                    # ---- Phase C: attention ----
                    with tc.tile_pool(name="pc_s", bufs=4,
                                      space="PSUM") as pcs, \
                         tc.tile_pool(name="pc_a", bufs=2,
                                      space="PSUM") as pca:
                        for i in range(NQB):
                            for head in range(NH):
                                qs = qT_sb[:, head, QBS * i:QBS * (i + 1)]
                                enc_ps = pca.tile([HD, QBS], F32, tag="enc_ps")
                                den_ps = pca.tile([1, QBS], F32, tag="den_ps")
                                ets = [None] * SLOTS

                                def emit_s(k):
                                    c0, c1, msk = SLOT_TRIM[k]
                                    w = c1 - c0
                                    s_ps = pcs.tile([128, QBS], F32,
                                                    tag="s_ps")
                                    kcol = 512 * i + 128 * k
                                    nc.tensor.matmul(
                                        out=s_ps[:, 0:w],
                                        lhsT=kT_sb[:, kcol:kcol + 128],
                                        rhs=qs[:, c0:c1],
                                        start=True, stop=msk is None)
                                    if msk is not None:
                                        toff = w - 128 if msk == "up" else 0
                                        nc.tensor.matmul(
                                            out=s_ps[:, toff:toff + 128],
                                            lhsT=ident[:, :],
                                            rhs=tri[:, 0 if msk == "up" else 1, :],
                                            start=False, stop=True,
                                            skip_group_check=True)
                                    blk = 4 * i + k
                                    # f32r et: ACT writes 16-bit outputs at
                                    # ~2/3 throughput, and f32r moving
                                    # operands >=256 cols run at full PE
                                    # speed anyway. Narrow (128-col) slots
                                    # stay bf16 to dodge the f32r 4x penalty.
                                    if w >= 256:
                                        et = pet.tile([128, QBS], F32R,
                                                      tag="et_w")
                                    else:
                                        et = pet.tile([128, 128], F32R,
                                                      tag="et_n", bufs=4)
                                    nc.scalar.activation(
                                        out=et[:, 0:w], in_=s_ps[:, 0:w],
                                        func=EXP,
                                        bias=padb[:, blk:blk + 1])
                                    ets[k] = et

                                def emit_acc(k, first, last):
                                    c0, c1, _ = SLOT_TRIM[k]
                                    w = c1 - c0
                                    blk = 4 * i + k
                                    nc.tensor.matmul(
                                        out=den_ps[:, c0:c1],
                                        lhsT=ones_sb[:, :],
                                        rhs=ets[k][:, 0:w],
                                        start=first, stop=last,
                                        skip_group_check=True)
                                    nc.tensor.matmul(
                                        out=enc_ps[:, c0:c1],
                                        lhsT=v_sb[:, 128 * blk:128 * (blk + 1)],
                                        rhs=ets[k][:, 0:w],
                                        start=first, stop=last,
                                        skip_group_check=True)

                                # software pipeline: S emitted LA slots ahead
                                LA = 4
                                for j in range(LA):
                                    emit_s(SLOT_ORDER[j])
                                for j in range(LA, SLOTS):
                                    emit_s(SLOT_ORDER[j])
                                    emit_acc(SLOT_ORDER[j - LA],
                                             j - LA == 0, False)
                                for j in range(SLOTS - LA, SLOTS):
                                    emit_acc(SLOT_ORDER[j], False,
                                             j == SLOTS - 1)

                                den_sb = pcn.tile([1, QBS], F32, tag="den_sb")
                                nc.vector.reciprocal_approx_fast(
                                    out=den_sb[:, :], in_=den_ps[:, :])
                                den_bc = pcn.tile([128, QBS], F32, tag="den_bc")
                                nc.gpsimd.partition_broadcast(
                                    den_bc[:, :], den_sb[:, :])
                                nc.vector.tensor_mul(
                                    encT[:, head, QBS * i:QBS * (i + 1)],
                                    enc_ps[:, :], den_bc[:, :])

                    # ---- Phase D: output projection ----
                    with tc.tile_pool(name="pdo", bufs=3) as pdo, \
                         tc.tile_pool(name="pd_ps", bufs=4,
                                      space="PSUM") as pdps:
                        for oc in range(4):
                            for tsub in range(QL // 128):
                                o_ps = pdps.tile([128, 512], F32, tag="o_ps")
                                for n in range(NH):
                                    nc.tensor.matmul(
                                        out=o_ps[:, :],
                                        lhsT=encT[:, n,
                                                  128 * tsub:128 * (tsub + 1)],
                                        rhs=wot_sb[:, oc, n, :],
                                        start=(n == 0), stop=(n == NH - 1))
                                o_sb = pdo.tile([128, 512], F32, tag="o_sb")
                                nc.vector.tensor_add(
                                    o_sb[:, :], o_ps[:, :],
                                    bias_bc[:, 512 * oc:512 * (oc + 1)])
                                nc.sync.dma_start(
                                    out=out[128 * tsub:128 * (tsub + 1),
                                            512 * oc:512 * (oc + 1)],
                                    in_=o_sb[:, :])

    nc.compile()
    return nc


def _get_program():
    global _COMPILED
    if _COMPILED is None:
        _COMPILED = _build_program()
    return _COMPILED


def _check_mask(attention_mask):
    """This kernel is specialized to the causal + sliding-window mask."""
    am = np.asarray(attention_mask)
    t = np.arange(T)[:, None]
    s = np.arange(T)[None, :]
    expect = (t >= s) & (t <= s + WIN)
    if am.shape != (T, T) or not np.array_equal(am, expect):
        raise ValueError(
            "attention_mask does not match the causal+window(1024) structure "
            "this kernel is specialized for")


def _prep_core_inputs(x, segment_pos, shared):
    """Per-core input dicts. Core c: batch c//2, query half c%2."""
    segment_pos = np.asarray(segment_pos)
    in_maps = []
    for c in range(8):
        b, h = c // 2, c % 2
        key_start = QL * h - WIN
        xb = x[b].T.astype(NPBF)                 # [W, T]
        # [halo | local] columns: key rows [key_start, key_start + KB),
        # zero-padded, chunked by width
        xtc = np.zeros((NW, 128, KB), dtype=NPBF)
        lo = max(0, -key_start)
        xtc[:, :, lo:] = xb[:, key_start + lo:key_start + KB] \
            .reshape(NW, 128, KB - lo)

        g_q = QL * h + np.arange(QL)                      # global query rows
        g_k = key_start + np.arange(KB)                   # global key rows
        pos_q = segment_pos[g_q]
        pos_k = np.where((g_k >= 0) & (g_k < T),
                         segment_pos[np.clip(g_k, 0, T - 1)], 0)
        cq, sq = _rope_tables(pos_q)
        ck, sk = _rope_tables(pos_k)

        ok_k = (g_k >= 0) & (g_k < T)
        padb = np.ascontiguousarray(np.where(
            ok_k, 0.0, NEG).astype(np.float32).reshape(KB // 128, 128).T)
        in_maps.append(dict(shared, xtc=xtc, cq=cq, sq=sq,
                            ck=ck, sk=sk, padb=padb))
    return in_maps


def _pack_w(wt):
    """[W, 2048] (transposed weight, contraction-major) ->
    [4, 128, 16, 512] stationary-chunk layout, bf16."""
    return np.ascontiguousarray(
        wt.reshape(NW, 128, 4, 512).transpose(2, 1, 0, 3)).astype(NPBF)


def kernel(x, segment_pos, attention_mask, wq, wk, wv, w_out, b_out):
    x = np.asarray(x, dtype=np.float32)
    wq = np.asarray(wq, dtype=np.float32)
    wk = np.asarray(wk, dtype=np.float32)
    wv = np.asarray(wv, dtype=np.float32)
    w_out = np.asarray(w_out, dtype=np.float32)
    b_out = np.asarray(b_out, dtype=np.float32)

    _check_mask(attention_mask)

    nc = _get_program()
    dt = np.arange(128)
    tri_up = np.where(dt[None, :] > dt[:, None], NEG, 0.0).astype(NPBF)
    tri_lo = np.where(dt[:, None] > dt[None, :], NEG, 0.0).astype(NPBF)
    shared = {
        "wqp": _pack_w(wq.T * np.float32(SCALE)),
        "wkp": np.ascontiguousarray(
            wk.T.reshape(NW, 128, HD).transpose(1, 0, 2)).astype(NPBF),
        "wvp": np.ascontiguousarray(
            wv.T.reshape(NW, 128, HD).transpose(1, 0, 2)).astype(NPBF),
        "wop": _pack_w(w_out.T),
        "bias": b_out,
        "tri": np.stack([tri_up, tri_lo]),
        "ident": np.eye(128, dtype=np.float32).astype(NPBF),
        "ones": np.ones((128, 1), dtype=np.float32),
    }
    in_maps = _prep_core_inputs(x, segment_pos, shared)
    res = run_bass_kernel_spmd(nc, in_maps, list(range(8)))
    global _LAST_RESULT
    _LAST_RESULT = res

    out = np.empty((B, T, W), dtype=np.float32)
    for c in range(8):
        b, h = c // 2, c % 2
        out[b, QL * h:QL * (h + 1), :] = res.results[c]["out"]
    return out
